# revision 49
# baseline (speedup 1.0000x reference)
"""Fused RoPE attention + LayerNorm, Trainium2, 8 NeuronCores (SPMD).

Sharding: every core takes the same 256-position slice of BOTH batches
(512 q-rows/core).  K/V projections are computed for the local rows,
all-gathered (fp8e4m3) across the 8 cores, then each core runs full
attention + LayerNorm for its rows.

Key optimizations over the v1 kernel:
- K and V all-gathers carry fp8e4m3 payloads (half the collective time;
  the AllGather floor here is ~13.5us + ~26us/MB).
- Scores matmuls are row-tiled: the two heads of a pair run concurrently
  on PE row-halves 0-63 / 64-127 via tile_position, doubling effective
  PE throughput for the DH=64 contraction.
- Softmax exp is split across engines: ScalarE does exact exp for most
  score tiles; the DVE computes a Schraudolph-style approximation
  (bf16 bit pattern = round(x * 184.66/8 + 16250.5) as int16, then
  bitcast) for a tunable subset, removing the single-engine exp
  bottleneck (16.8M exps/core).
- V values ride the AllGather with an interleaved ones column so the AV
  matmul emits softmax denominators for free (65-row transposed attn).
"""
import sys
import types
import os
import numpy as np
from contextlib import ExitStack

for _p in ("/opt/trn_rl_repo",):
    if _p not in sys.path:
        sys.path.append(_p)

# NTFF profile hook shim: lets BASS_TRACE=1 work in images whose antenv
# lacks axon_hooks (bass_utils imports it when tracing under axon).
if "antenv.axon_hooks" not in sys.modules:
    _hooks = types.ModuleType("antenv.axon_hooks")
    _HOOK = [None]
    _hooks.set_axon_ntff_profile_hook = lambda h: _HOOK.__setitem__(0, h)
    _hooks.get_axon_ntff_profile_hook = lambda: _HOOK[0]
    sys.modules["antenv.axon_hooks"] = _hooks
    try:
        from trn_agent_boot.trn_boot import _ntff_profile_via_ctypes

        _HOOK[0] = _ntff_profile_via_ctypes("/opt/axon/libaxon_pjrt.so")
    except Exception:
        pass

import concourse.bass as bass  # noqa: E402
import concourse.bacc as bacc  # noqa: E402
import concourse.mybir as mybir  # noqa: E402
import concourse.tile as tile  # noqa: E402
from concourse import bass_utils  # noqa: E402

F32 = mybir.dt.float32
BF16 = mybir.dt.bfloat16
I16 = mybir.dt.int16
FP8 = mybir.dt.float8e4
FP8E3 = mybir.dt.float8e3
NP_BF16 = np.dtype(mybir.dt.np(BF16))
NP_FP8 = np.dtype(mybir.dt.np(FP8))
AF = mybir.ActivationFunctionType
ALU = mybir.AluOpType
AX = mybir.AxisListType

B, S, D, H, DH = 2, 2048, 1024, 16, 64
NC = 8
SL = S // NC          # 256 positions per core (per batch)
R = B * SL            # 512 rows per core
G = H // 2            # 8 head-pairs
DC = D // 128         # 8 contraction chunks
KT = S // 128         # 16 k-tiles per batch
LN_EPS = 1e-5
ROPE_BASE = 10000.0

# Averaged-Schraudolph exp on the DVE (rounds RNE on f32->i16 output):
#   j1 = round(y*A + B1);  j2 = j1 - 65
#   exp(s) ~= bitcast16(j1) + bitcast16(j2)
# The -65 bit offset simultaneously provides the half-period phase
# shift AND the 2^(-65/128) weight of the second sample, so the two
# sawtooth error terms cancel to ~1.4% max (vs 3.3% single-sample)
# with a plain tensor_tensor add as the combine.
SCH_A = 184.6649652337873 * 0.125   # folds in the 1/sqrt(DH) scale
SCH_B1 = 16151.0
SCH_OFF = 65.0

# Which (chunk, hh) exp tiles go to the DVE (avg-Schraudolph); rest go
# to ScalarE (exact).  8 tiles per block: (c,hh), c in 0..3, hh in 0..1.
_KSCH = int(os.environ.get("KSCH", "2"))
DVE_TILES = [(), ((1, 1),), ((1, 1), (3, 1)),
             ((1, 0), (1, 1), (3, 1)),
             ((1, 0), (1, 1), (3, 0), (3, 1))][min(_KSCH, 4)]
DVE_TILES = set(DVE_TILES)
K_E3 = os.environ.get("K_E3", "1") == "1"    # K allgather in fp8-e3m4
ROWTILE = os.environ.get("ROWTILE", "1") == "1"
# AV runs KLAG blocks behind scores/exp so its matmuls never head-of-
# line-block the PE queue while the V allgathers are still in flight.
KLAG = int(os.environ.get("KLAG", "3"))


def _build(flags):
    has_bqk, has_bv, has_gb = flags
    K_DT = FP8E3 if K_E3 else BF16
    V_DT = BF16
    nc = bacc.Bacc("TRN2", target_bir_lowering=False, debug=False,
                   num_devices=NC)

    xqT = nc.dram_tensor("xqT", [D, R], BF16, kind="ExternalInput")
    xvT = nc.dram_tensor("xvT", [D, R], BF16, kind="ExternalInput")
    wq_d = nc.dram_tensor("wq", [D, D], BF16, kind="ExternalInput")
    wk_d = nc.dram_tensor("wk", [D, D], BF16, kind="ExternalInput")
    wv_d = nc.dram_tensor("wv", [D, D], BF16, kind="ExternalInput")
    perm_d = nc.dram_tensor("perm", [128, 128], BF16, kind="ExternalInput")
    ident_d = nc.dram_tensor("ident", [128, 128], BF16, kind="ExternalInput")
    cos_d = nc.dram_tensor("cos", [128, R], F32, kind="ExternalInput")
    sin_d = nc.dram_tensor("sin", [128, R], F32, kind="ExternalInput")
    if has_bqk:
        cq_d = nc.dram_tensor("cq", [D, R], F32, kind="ExternalInput")
        ck_d = nc.dram_tensor("ck", [D, R], F32, kind="ExternalInput")
    if has_bv:
        bv_d = nc.dram_tensor("bv", [128, D], F32, kind="ExternalInput")
    if has_gb:
        gam_d = nc.dram_tensor("gamma", [128, D], F32, kind="ExternalInput")
        bet_d = nc.dram_tensor("beta", [128, D], F32, kind="ExternalInput")
    out_d = nc.dram_tensor("out", [R, D], F32, kind="ExternalOutput")

    es = ExitStack()
    with es:
        tc = es.enter_context(tile.TileContext(nc))
        dram = es.enter_context(
            tc.tile_pool(name="dram", bufs=1, space="DRAM"))
        constp = es.enter_context(tc.tile_pool(name="const", bufs=1))
        qp = es.enter_context(tc.tile_pool(name="qp", bufs=1))
        kvs = es.enter_context(tc.tile_pool(name="kvs", bufs=8))
        attnp = es.enter_context(tc.tile_pool(name="attnp", bufs=1))
        epip = es.enter_context(tc.tile_pool(name="epip", bufs=8))
        lnp = es.enter_context(tc.tile_pool(name="lnp", bufs=2))
        outp = es.enter_context(tc.tile_pool(name="outp", bufs=2))

        # K allgather split by batch: batch 0's K lands first and unblocks
        # the (batch-outer-ordered) attention blocks ~10us earlier.
        bounce_kb = [dram.tile([D, SL], K_DT, tag=f"bkb{b}",
                               name=f"bkb{b}") for b in range(B)]
        ag_kb = [dram.tile([NC * D, SL], K_DT, tag=f"agkb{b}",
                           name=f"agkb{b}", addr_space="Shared")
                 for b in range(B)]
        bounce_v = dram.tile([R, H * 65], V_DT, tag="bv")
        # V allgather is split by batch so AV for batch 0 can start while
        # batch 1's shards are still in flight.
        ag_vb = [dram.tile([NC * SL, H * 65], V_DT, tag=f"agv{b}",
                           name=f"agv{b}", addr_space="Shared")
                 for b in range(B)]

        cos_sb = constp.tile([128, R], F32, tag="cos")
        sin_sb = constp.tile([128, R], F32, tag="sin")
        perm_sb = constp.tile([128, 128], BF16, tag="perm")
        ident_sb = constp.tile([128, 128], BF16, tag="ident")
        eps_sb = constp.tile([128, 1], F32, tag="eps")
        nc.vector.memset(eps_sb[:], LN_EPS)
        cq_sb = ck_sb = bv_sb = gam_sb = bet_sb = None
        if has_bqk:
            cq_sb = constp.tile([128, DC * R], F32, tag="cq")
            ck_sb = constp.tile([128, DC * R], F32, tag="ck")
            for g in range(G):
                nc.sync.dma_start(cq_sb[:, g * R:(g + 1) * R],
                                  cq_d[g * 128:(g + 1) * 128, :])
                nc.sync.dma_start(ck_sb[:, g * R:(g + 1) * R],
                                  ck_d[g * 128:(g + 1) * 128, :])
        if has_bv:
            bv_sb = constp.tile([128, D], F32, tag="bvs")
            nc.sync.dma_start(bv_sb[:], bv_d[:])
        if has_gb:
            gam_sb = constp.tile([128, D], F32, tag="gam")
            nc.sync.dma_start(gam_sb[:], gam_d[:])
            bet_sb = constp.tile([128, D], F32, tag="bet")
            nc.sync.dma_start(bet_sb[:], bet_d[:])

        q_sb = qp.tile([128, G * R], BF16, tag="qrot")

        pes = ExitStack()
        with pes:
            xp = pes.enter_context(tc.tile_pool(name="xp", bufs=1))
            wp = pes.enter_context(tc.tile_pool(name="wp", bufs=2))
            stage = pes.enter_context(tc.tile_pool(name="stage", bufs=3))
            usbp = pes.enter_context(tc.tile_pool(name="usbp", bufs=3))
            krotp = pes.enter_context(tc.tile_pool(name="krotp", bufs=2))
            vstp = pes.enter_context(tc.tile_pool(name="vstp", bufs=2))
            pjp = pes.enter_context(
                tc.tile_pool(name="pjp", bufs=4, space="PSUM"))
            pvp = pes.enter_context(
                tc.tile_pool(name="pvp", bufs=2, space="PSUM"))

            def load_w(t_dram):
                w_sb = wp.tile([128, DC * D], BF16, tag="w")
                for dc in range(DC):
                    nc.sync.dma_start(w_sb[:, dc * D:(dc + 1) * D],
                                      t_dram[dc * 128:(dc + 1) * 128, :])
                return w_sb

            # K first: its all-gather is on the attention critical path.
            wk_sb = load_w(wk_d)
            xq_sb = xp.tile([128, DC * R], BF16, tag="xq")
            for dc in range(DC):
                nc.sync.dma_start(xq_sb[:, dc * R:(dc + 1) * R],
                                  xqT[dc * 128:(dc + 1) * 128, :])
            nc.sync.dma_start(perm_sb[:], perm_d[:])
            nc.sync.dma_start(cos_sb[:], cos_d[:])
            nc.sync.dma_start(sin_sb[:], sin_d[:])
            nc.sync.dma_start(ident_sb[:], ident_d[:])
            xv_sb = xp.tile([128, DC * R], BF16, tag="xv")
            for dc in range(DC):
                nc.sync.dma_start(xv_sb[:, dc * R:(dc + 1) * R],
                                  xvT[dc * 128:(dc + 1) * 128, :])

            # --- Q/K projection, software-pipelined so the perm matmul of
            # group g runs behind the U matmuls of group g+1. ---
            def proj_u(w_sb, g):
                ps_u = pjp.tile([128, R], F32, tag="pj",
                                name=f"psu{id(w_sb)}_{g}")
                for dc in range(DC):
                    nc.tensor.matmul(
                        ps_u[:],
                        w_sb[:, dc * D + g * 128: dc * D + (g + 1) * 128],
                        xq_sb[:, dc * R:(dc + 1) * R],
                        start=(dc == 0), stop=(dc == DC - 1))
                u_sb = usbp.tile([128, R], BF16, tag="usb",
                                 name=f"usb{id(w_sb)}_{g}")
                nc.scalar.copy(u_sb[:], ps_u[:])
                return ps_u, u_sb

            def proj_rope(g, ps_u, u_sb, c_sb, dst):
                ps_u2 = pjp.tile([128, R], F32, tag="pj", name=f"psu2_{g}")
                nc.tensor.matmul(ps_u2[:], perm_sb[:], u_sb[:],
                                 start=True, stop=True)
                t1 = stage.tile([128, R], F32, tag="st", name=f"t1_{g}")
                nc.vector.tensor_mul(t1[:], ps_u[:], cos_sb[:])
                t2 = stage.tile([128, R], F32, tag="st", name=f"t2_{g}")
                nc.vector.tensor_mul(t2[:], ps_u2[:], sin_sb[:])
                if c_sb is None:
                    nc.vector.tensor_add(dst, t1[:], t2[:])
                else:
                    t3 = stage.tile([128, R], F32, tag="st", name=f"t3_{g}")
                    nc.vector.tensor_add(t3[:], t1[:], t2[:])
                    nc.vector.tensor_add(
                        dst, t3[:], c_sb[:, g * R:(g + 1) * R])

            def qk_proj_all(w_sb, c_sb, emit_dst, tail, groups):
                pend = None
                for g in groups:
                    cur = (g,) + proj_u(w_sb, g)
                    if pend is not None:
                        gp = pend[0]
                        proj_rope(*pend, c_sb, emit_dst(gp))
                        tail(gp)
                    pend = cur
                gp = pend[0]
                proj_rope(*pend, c_sb, emit_dst(gp))
                tail(gp)

            # K projection + RoPE -> bounce (fp8), single AllGather
            krots = {}

            def k_dst(g):
                krots[g] = krotp.tile([128, R], K_DT, tag="kr",
                                      name=f"kr{g}")
                return krots[g][:]

            def k_tail(g):
                for b in range(B):
                    nc.sync.dma_start(
                        bounce_kb[b][g * 128:(g + 1) * 128, :],
                        krots[g][:, b * SL:(b + 1) * SL])

            qk_proj_all(wk_sb, ck_sb, k_dst, k_tail, range(G))
            nc.gpsimd.collective_compute(
                "AllGather", ALU.bypass,
                ins=[bounce_kb[0][:].opt()], outs=[ag_kb[0][:].opt()],
                replica_groups=[list(range(NC))])

            # V projection -> bounce (fp8, ones interleaved), AllGather
            wv_sb = load_w(wv_d)
            for st in range(R // 128):
                ps_v = pvp.tile([128, D], F32, tag="pv")
                for dc in range(DC):
                    for hf in range(2):
                        nc.tensor.matmul(
                            ps_v[:, hf * 512:(hf + 1) * 512],
                            xv_sb[:, dc * R + st * 128:
                                  dc * R + st * 128 + 128],
                            wv_sb[:, dc * D + hf * 512:
                                  dc * D + (hf + 1) * 512],
                            start=(dc == 0), stop=(dc == DC - 1))
                if has_bv:
                    nc.vector.tensor_add(ps_v[:], ps_v[:], bv_sb[:])
                v_sb = vstp.tile([128, H * 65], V_DT, tag="vst")
                v3 = v_sb[:].rearrange("p (h e) -> p h e", e=65)
                nc.vector.memset(v3[:, :, 64:65], 1.0)
                nc.scalar.copy(
                    v3[:, :, 0:64],
                    ps_v[:].rearrange("p (h d) -> p h d", d=64))
                nc.sync.dma_start(
                    bounce_v[st * 128:(st + 1) * 128, :], v_sb[:])
                if st % 2 == 1:
                    b = st // 2
                    nc.gpsimd.collective_compute(
                        "AllGather", ALU.bypass,
                        ins=[bounce_v[b * SL:(b + 1) * SL, :].opt()],
                        outs=[ag_vb[b][:].opt()],
                        replica_groups=[list(range(NC))])
                    if b == 0:
                        # batch 1's K chains warm between V0 and V1
                        nc.gpsimd.collective_compute(
                            "AllGather", ALU.bypass,
                            ins=[bounce_kb[1][:].opt()],
                            outs=[ag_kb[1][:].opt()],
                            replica_groups=[list(range(NC))])

            # Q projection + RoPE (stays local).
            wq_sb = load_w(wq_d)

            def q_dst(g):
                return q_sb[:, g * R:(g + 1) * R]

            qk_proj_all(wq_sb, cq_sb, q_dst, lambda g: None, range(G))

        kph_all = {}

        def load_pair(g, b):
            # one batch-half of one head-pair's K, reloaded per block
            kph = kvs.tile([128, S], K_DT, tag="kp", name=f"kp{g}_{b}")
            for r in range(NC):
                srcap = ag_kb[b][r * D + g * 128: r * D + (g + 1) * 128, :]
                nc.sync.dma_start(kph[:, r * SL:(r + 1) * SL], srcap)
            kph_all[(g, b)] = kph

        # pts pool is created after the projection pools are released so
        # the KLAG-deep prob tiles reuse that SBUF.
        ptp = es.enter_context(
            tc.tile_pool(name="ptp", bufs=8 * (KLAG + 1) + 2))

        # prefetch block 0's K before the bulk v_full loads hit the queues
        load_pair(0, 0)

        # V resident for the whole attention phase: [s-tile, 16 heads x 65]
        # per (batch, k-tile), contiguous lines.  The loads are emitted
        # lazily inside the block loop (load_v below) so the descriptors
        # don't sit in the DMA queues blocking per-block K loads while
        # the V allgather is still in flight.
        vfp = es.enter_context(tc.tile_pool(name="vfp", bufs=1))
        v_full = vfp.tile([128, B * KT * H * 65], V_DT, tag="vfull")

        def load_v(b, kt):
            nc.sync.dma_start(
                v_full[:, (b * KT + kt) * (H * 65):
                       (b * KT + kt + 1) * (H * 65)],
                ag_vb[b][kt * 128:(kt + 1) * 128, :])

        attn_sb = [attnp.tile([128, D], F32, tag=f"attn{t}", name=f"attn{t}")
                   for t in range(4)]

        aes = ExitStack()
        with aes:
            scp = aes.enter_context(
                tc.tile_pool(name="scp", bufs=3, space="PSUM"))
            avp = aes.enter_context(
                tc.tile_pool(name="avp", bufs=1, space="PSUM"))
            trp = aes.enter_context(
                tc.tile_pool(name="trp", bufs=1, space="PSUM"))
            atsb = aes.enter_context(tc.tile_pool(name="atsb", bufs=3))
            schp = aes.enter_context(tc.tile_pool(name="schp", bufs=2))

            def emit_scores(g, b, grp):
                kph = kph_all[(g, b)]
                ps_s = [scp.tile([128, 1024], F32, tag="sc",
                                 name=f"pss{g}_{b}_{grp}_{_i}")
                        for _i in range(2)]
                for jj in range(4):
                    kt = grp * 4 + jj
                    for hh in range(2):
                        nc.tensor.matmul(
                            ps_s[hh][:, jj * SL:(jj + 1) * SL],
                            kph[hh * 64:(hh + 1) * 64,
                                kt * 128:(kt + 1) * 128],
                            q_sb[hh * 64:(hh + 1) * 64,
                                 g * R + b * SL:
                                 g * R + (b + 1) * SL],
                            start=True, stop=True,
                            tile_position=(hh * 64, 0) if ROWTILE
                            else None)
                return ps_s

            def emit_exp(g, b, grp, hh, ps, pts):
                if (grp, hh) in DVE_TILES:
                    e1 = schp.tile([128, 1024], I16, tag="e1",
                                   name=f"e1_{g}_{b}_{grp}_{hh}")
                    nc.vector.tensor_scalar(
                        e1[:], ps[:], SCH_A, SCH_B1, ALU.mult, ALU.add)
                    e2 = schp.tile([128, 1024], I16, tag="e2",
                                   name=f"e2_{g}_{b}_{grp}_{hh}")
                    nc.vector.tensor_scalar(
                        e2[:], e1[:], SCH_OFF, None, ALU.subtract)
                    pt = ptp.tile([128, 1024], BF16, tag="pt",
                                  name=f"pt{g}_{b}_{grp}_{hh}")
                    nc.vector.tensor_add(
                        pt[:], e1[:].bitcast(BF16), e2[:].bitcast(BF16))
                else:
                    pt = ptp.tile([128, 1024], BF16, tag="pt",
                                  name=f"pt{g}_{b}_{grp}_{hh}")
                    nc.scalar.activation(
                        pt[:], ps[:], AF.Exp, scale=0.125)
                pts[(grp, hh)] = pt[:]

            def emit_av_quarter(g, b, pts, aTp, grp):
                # attn^T accumulation: out[65, 256] = [V_h | 1]^T @ P^T,
                # column-tiled (32+32+1) so the three matmuls run on
                # disjoint PE column-groups concurrently and their
                # LDWEIGHTS pull ahead (an untiled 65-col stationary
                # serializes LDW+MM at ~166ns per k-tile).
                # start=True clears has_written for the WHOLE bank, so it
                # may only appear on the block's very first AV matmul.
                for hh in range(2):
                    h = 2 * g + hh
                    for jj in range(4):
                        kt = grp * 4 + jj
                        vbase = (b * KT + kt) * (H * 65) + h * 65
                        mv = pts[(grp, hh)][:, jj * SL:(jj + 1) * SL]
                        first = (kt == 0 and hh == 0)
                        last = (kt == 15 and hh == 1)
                        for (c0, c1, tp) in ((0, 32, 0), (32, 64, 32),
                                             (64, 65, 64)):
                            nc.tensor.matmul(
                                aTp[c0:c1, hh * SL:(hh + 1) * SL],
                                v_full[:, vbase + c0: vbase + c1],
                                mv,
                                start=first, stop=last,
                                tile_position=(0, tp),
                                skip_group_check=True)

            def emit_cast(g, b, aTp):
                aT_sb = atsb.tile([65, 2 * SL], BF16, tag="ats",
                                  name=f"ats{g}_{b}")
                nc.vector.tensor_copy(aT_sb[:], aTp[:])
                return aT_sb

            def emit_tr_norm(g, b, aT_sb):
                # PE-transpose attn^T back to [q, dh], then normalize by
                # the gathered denominators (65th row).
                tr = trp.tile([128, 4 * 66], BF16, tag="tr",
                              name=f"tr{g}_{b}")
                for hh in range(2):
                    for t in range(2):
                        idx = hh * 2 + t
                        nc.tensor.transpose(
                            tr[:, idx * 66: idx * 66 + 65],
                            aT_sb[:, hh * SL + t * 128:
                                  hh * SL + (t + 1) * 128],
                            ident_sb[0:65, 0:65])
                rec = epip.tile([128, 4], F32, tag="rec",
                                name=f"rec{g}_{b}")
                nc.vector.reciprocal(rec[:], tr[:, 64::66])
                for hh in range(2):
                    h = 2 * g + hh
                    for t in range(2):
                        idx = hh * 2 + t
                        qtg = b * 2 + t
                        nc.vector.tensor_scalar(
                            attn_sb[qtg][:, h * 64:(h + 1) * 64],
                            tr[:, idx * 66: idx * 66 + 64],
                            rec[:, idx: idx + 1], None, ALU.mult)

            def emit_fixups(g, b, aTp):
                emit_tr_norm(g, b, emit_cast(g, b, aTp))

            # KLAG-block software pipeline with quarter-grain interleave:
            # the AV matmuls of block i-KLAG are emitted between the
            # score chunk-pairs of block i, so the PE always has
            # independent, dependency-satisfied work queued while the exp
            # stream paces the pipeline (and the HAM clock gate stays
            # warm).  Batch-outer block order: the V allgather for batch
            # b lands well before block b*G+KLAG needs it.
            blocks = [(g, b) for b in range(B) for g in range(G)]
            pend = []   # (g, b, pts) awaiting AV, oldest first

            def do_av(g, b, pts):
                aTp = avp.tile([65, 2 * SL], F32, tag="av",
                               name=f"aT{g}_{b}")
                for grp in range(4):
                    emit_av_quarter(g, b, pts, aTp, grp)
                return aTp

            fixq = []   # (g, b, aT_sb) cast done, awaiting transpose+norm
            for i, (g, b) in enumerate(blocks):
                fix_old = fixq.pop(0) if fixq else None
                if i + 1 < len(blocks):
                    load_pair(*blocks[i + 1])
                # stream the v_full loads: batch 0's over blocks 0-3
                # (complete before their first AV consumer at block KLAG),
                # batch 1's over blocks 7-10 (emitted before their block-
                # (8+KLAG) consumer but late enough that the descriptors
                # don't sit in the DMA queues waiting on the V1 allgather)
                if i < 4:
                    for kt in range(4 * i, 4 * i + 4):
                        load_v(0, kt)
                elif 7 <= i < 11:
                    for kt in range(4 * (i - 7), 4 * (i - 7) + 4):
                        load_v(1, kt)
                pts = {}
                aT_prev = None
                old = pend.pop(0) if len(pend) >= KLAG else None
                for grp in range(4):
                    ps_s = emit_scores(g, b, grp)
                    if old is not None:
                        if grp == 0:
                            aT_prev = avp.tile(
                                [65, 2 * SL], F32, tag="av",
                                name=f"aT{old[0]}_{old[1]}")
                        emit_av_quarter(old[0], old[1], old[2],
                                        aT_prev, grp)
                    if grp == 2 and fix_old is not None:
                        # one block after its AV: PE transposes + DVE
                        # normalize, with inputs long since ready (no
                        # head-of-line stalls in any engine queue)
                        emit_tr_norm(fix_old[0], fix_old[1], fix_old[2])
                    emit_exp(g, b, grp, 0, ps_s[0], pts)
                    emit_exp(g, b, grp, 1, ps_s[1], pts)
                if old is not None:
                    # evacuate aT promptly (frees the single avp buffer
                    # for the next block's AV quarters)
                    fixq.append((old[0], old[1],
                                 emit_cast(old[0], old[1], aT_prev)))
                # drain an extra pending AV near the end so the tail after
                # the last exp is short
                if i >= len(blocks) - KLAG + 1 and pend:
                    g2, b2, pts2 = pend.pop(0)
                    aTp2 = do_av(g2, b2, pts2)
                    fixq.append((g2, b2, emit_cast(g2, b2, aTp2)))
                pend.append((g, b, pts))
            for (g, b, pts) in pend:
                aTp = do_av(g, b, pts)
                fixq.append((g, b, emit_cast(g, b, aTp)))
            for (g, b, aT_sb) in fixq:
                emit_tr_norm(g, b, aT_sb)

        # --- LayerNorm over D (var = E[x^2] - mu^2; square+row-sum on the
        # ScalarE accumulator) + store ---
        for qtg in range(4):
            at = attn_sb[qtg]
            sums = epip.tile([128, 1], F32, tag="s1", name=f"s1_{qtg}")
            nc.vector.reduce_sum(sums[:], at[:], axis=AX.X)
            sq = lnp.tile([128, D], F32, tag="sq", name=f"sq{qtg}")
            ssum = epip.tile([128, 1], F32, tag="s3", name=f"s3_{qtg}")
            nc.scalar.activation(sq[:], at[:], AF.Square,
                                 accum_out=ssum[:])
            mu = epip.tile([128, 1], F32, tag="s2", name=f"s2_{qtg}")
            nc.vector.tensor_scalar_mul(mu[:], sums[:], 1.0 / D)
            var = epip.tile([128, 1], F32, tag="s6", name=f"s6_{qtg}")
            nc.vector.tensor_scalar(var[:], mu[:], mu[:], None, ALU.mult)
            nc.vector.tensor_scalar(
                var[:], var[:], -1.0, None, ALU.mult)
            nc.vector.scalar_tensor_tensor(
                var[:], ssum[:], 1.0 / D, var[:], ALU.mult, ALU.add)
            std = epip.tile([128, 1], F32, tag="s4", name=f"s4_{qtg}")
            nc.scalar.activation(std[:], var[:], AF.Sqrt, bias=eps_sb[:])
            rstd = epip.tile([128, 1], F32, tag="s5", name=f"s5_{qtg}")
            nc.vector.reciprocal(rstd[:], std[:])
            mrs = epip.tile([128, 1], F32, tag="s7", name=f"s7_{qtg}")
            nc.vector.tensor_scalar(mrs[:], mu[:], rstd[:], None, ALU.mult)
            o_sb = outp.tile([128, D], F32, tag="o", name=f"oo{qtg}")
            nc.vector.tensor_scalar(
                o_sb[:], at[:], rstd[:], mrs[:], ALU.mult, ALU.subtract)
            if has_gb:
                nc.vector.tensor_mul(o_sb[:], o_sb[:], gam_sb[:])
                nc.vector.tensor_add(o_sb[:], o_sb[:], bet_sb[:])
            nc.sync.dma_start(out_d[qtg * 128:(qtg + 1) * 128, :], o_sb[:])

    nc.compile()
    return nc


_CACHE: dict = {}
LAST_EXEC_NS = None


def _rope_tables():
    half = DH // 2
    inv_freq = 1.0 / (ROPE_BASE ** (np.arange(half, dtype=np.float32) / half))
    t = np.arange(S, dtype=np.float32)
    freqs = t[:, None] * inv_freq[None, :]
    emb = np.concatenate([freqs, freqs], axis=-1)          # [S, DH]
    return np.cos(emb).astype(np.float32), np.sin(emb).astype(np.float32)


def prep_flags(inputs):
    b_qk = np.asarray(inputs["b_qk"], dtype=np.float32)
    b_v = np.asarray(inputs["b_v"], dtype=np.float32)
    gamma = np.asarray(inputs["ln_gamma"], dtype=np.float32)
    beta = np.asarray(inputs["ln_beta"], dtype=np.float32)
    return (bool(np.any(b_qk)), bool(np.any(b_v)),
            bool(np.any(gamma != 1.0) or np.any(beta != 0.0)))


def _prep_in_maps(inputs, flags):
    x_qk = np.asarray(inputs["x_qk"], dtype=np.float32)
    x_v = np.asarray(inputs["x_v"], dtype=np.float32)
    W_qk = np.asarray(inputs["W_qk"], dtype=np.float32)
    b_qk = np.asarray(inputs["b_qk"], dtype=np.float32)
    W_v = np.asarray(inputs["W_v"], dtype=np.float32)
    b_v = np.asarray(inputs["b_v"], dtype=np.float32)
    gamma = np.asarray(inputs["ln_gamma"], dtype=np.float32)
    beta = np.asarray(inputs["ln_beta"], dtype=np.float32)

    # signed pair-swap: rot2(v)[j] = sum_l Pm[l, j] v[l]
    Pm = np.zeros((128, 128), np.float32)
    for i in range(64):
        Pm[2 * i + 1, 2 * i] = -1.0
        Pm[2 * i, 2 * i + 1] = 1.0
    Pm64 = Pm[:DH, :DH]

    cos_all, sin_all = _rope_tables()
    Wq = W_qk[:, :D]
    Wk = W_qk[:, D:]
    bq = b_qk[:D]
    bk = b_qk[D:]
    bq2 = (bq.reshape(H, DH) @ Pm64).reshape(D)
    bk2 = (bk.reshape(H, DH) @ Pm64).reshape(D)

    wq_np = np.ascontiguousarray(Wq.astype(NP_BF16))
    wk_np = np.ascontiguousarray(Wk.astype(NP_BF16))
    wv_np = np.ascontiguousarray(W_v.astype(NP_BF16))
    perm_np = np.ascontiguousarray(Pm.astype(NP_BF16))

    xf = x_qk.reshape(B * S, D)
    xvf = x_v.reshape(B * S, D)

    in_maps = []
    for c in range(NC):
        ps = np.arange(SL * c, SL * (c + 1))
        rows = np.concatenate([ps, S + ps])          # both batches
        xqT_c = np.ascontiguousarray(xf[rows].T.astype(NP_BF16))
        xvT_c = np.ascontiguousarray(xvf[rows].T.astype(NP_BF16))
        cos_c = np.ascontiguousarray(np.tile(cos_all[ps].T, (2, 2)))
        sin_c = np.ascontiguousarray(np.tile(sin_all[ps].T, (2, 2)))
        m = {
            "xqT": xqT_c, "xvT": xvT_c,
            "wq": wq_np, "wk": wk_np, "wv": wv_np,
            "perm": perm_np, "cos": cos_c, "sin": sin_c,
            "ident": np.ascontiguousarray(np.eye(128, dtype=NP_BF16)),
        }
        if flags[0]:
            cos_f = np.tile(cos_all[ps].T, (H, 2))   # [1024, 512]
            sin_f = np.tile(sin_all[ps].T, (H, 2))
            m["cq"] = np.ascontiguousarray(
                bq[:, None] * cos_f + bq2[:, None] * sin_f)
            m["ck"] = np.ascontiguousarray(
                bk[:, None] * cos_f + bk2[:, None] * sin_f)
        if flags[1]:
            m["bv"] = np.ascontiguousarray(
                np.broadcast_to(b_v, (128, D)).astype(np.float32))
        if flags[2]:
            m["gamma"] = np.ascontiguousarray(
                np.broadcast_to(gamma, (128, D)).astype(np.float32))
            m["beta"] = np.ascontiguousarray(
                np.broadcast_to(beta, (128, D)).astype(np.float32))
        in_maps.append(m)
    return in_maps


def assemble_output(per_core_outs):
    out = np.empty((B * S, D), np.float32)
    for c in range(NC):
        oc = np.asarray(per_core_outs[c], dtype=np.float32)
        for b in range(B):
            out[b * S + SL * c: b * S + SL * (c + 1)] = \
                oc[b * SL:(b + 1) * SL]
    return out.reshape(B, S, D)


def kernel(**inputs):
    flags = prep_flags(inputs)
    if flags not in _CACHE:
        _CACHE[flags] = _build(flags)
    nc = _CACHE[flags]
    in_maps = _prep_in_maps(inputs, flags)
    res = bass_utils.run_bass_kernel_spmd(
        nc, in_maps, core_ids=list(range(NC)))
    global LAST_EXEC_NS
    LAST_EXEC_NS = res.exec_time_ns
    return assemble_output([res.results[c]["out"] for c in range(NC)])


# revision 50
# speedup vs baseline: 1.1077x; 1.1077x over previous
"""Fused RoPE attention + LayerNorm, Trainium2, 8 NeuronCores (SPMD).

Sharding: every core takes the same 256-position slice of BOTH batches
(512 q-rows/core).  K/V projections are computed for the local rows,
all-gathered (fp8e4m3) across the 8 cores, then each core runs full
attention + LayerNorm for its rows.

Key optimizations over the v1 kernel:
- K and V all-gathers carry fp8e4m3 payloads (half the collective time;
  the AllGather floor here is ~13.5us + ~26us/MB).
- Scores matmuls are row-tiled: the two heads of a pair run concurrently
  on PE row-halves 0-63 / 64-127 via tile_position, doubling effective
  PE throughput for the DH=64 contraction.
- Softmax exp is split across engines: ScalarE does exact exp for most
  score tiles; the DVE computes a Schraudolph-style approximation
  (bf16 bit pattern = round(x * 184.66/8 + 16250.5) as int16, then
  bitcast) for a tunable subset, removing the single-engine exp
  bottleneck (16.8M exps/core).
- V values ride the AllGather with an interleaved ones column so the AV
  matmul emits softmax denominators for free (65-row transposed attn).
"""
import sys
import types
import os
import numpy as np
from contextlib import ExitStack

for _p in ("/opt/trn_rl_repo",):
    if _p not in sys.path:
        sys.path.append(_p)

# NTFF profile hook shim: lets BASS_TRACE=1 work in images whose antenv
# lacks axon_hooks (bass_utils imports it when tracing under axon).
if "antenv.axon_hooks" not in sys.modules:
    _hooks = types.ModuleType("antenv.axon_hooks")
    _HOOK = [None]
    _hooks.set_axon_ntff_profile_hook = lambda h: _HOOK.__setitem__(0, h)
    _hooks.get_axon_ntff_profile_hook = lambda: _HOOK[0]
    sys.modules["antenv.axon_hooks"] = _hooks
    try:
        from trn_agent_boot.trn_boot import _ntff_profile_via_ctypes

        _HOOK[0] = _ntff_profile_via_ctypes("/opt/axon/libaxon_pjrt.so")
    except Exception:
        pass

import concourse.bass as bass  # noqa: E402
import concourse.bacc as bacc  # noqa: E402
import concourse.mybir as mybir  # noqa: E402
import concourse.tile as tile  # noqa: E402
from concourse import bass_utils  # noqa: E402

F32 = mybir.dt.float32
BF16 = mybir.dt.bfloat16
I16 = mybir.dt.int16
FP8 = mybir.dt.float8e4
FP8E3 = mybir.dt.float8e3
NP_BF16 = np.dtype(mybir.dt.np(BF16))
NP_FP8 = np.dtype(mybir.dt.np(FP8))
AF = mybir.ActivationFunctionType
ALU = mybir.AluOpType
AX = mybir.AxisListType

B, S, D, H, DH = 2, 2048, 1024, 16, 64
NC = 8
SL = S // NC          # 256 positions per core (per batch)
R = B * SL            # 512 rows per core
G = H // 2            # 8 head-pairs
DC = D // 128         # 8 contraction chunks
KT = S // 128         # 16 k-tiles per batch
LN_EPS = 1e-5
ROPE_BASE = 10000.0

# Averaged-Schraudolph exp on the DVE (rounds RNE on f32->i16 output):
#   j1 = round(y*A + B1);  j2 = j1 - 65
#   exp(s) ~= bitcast16(j1) + bitcast16(j2)
# The -65 bit offset simultaneously provides the half-period phase
# shift AND the 2^(-65/128) weight of the second sample, so the two
# sawtooth error terms cancel to ~1.4% max (vs 3.3% single-sample)
# with a plain tensor_tensor add as the combine.
SCH_A = 184.6649652337873 * 0.125   # folds in the 1/sqrt(DH) scale
SCH_B1 = 16151.0
SCH_OFF = 65.0

# Which (chunk, hh) exp tiles go to the DVE (avg-Schraudolph); rest go
# to ScalarE (exact).  8 tiles per block: (c,hh), c in 0..3, hh in 0..1.
_KSCH = int(os.environ.get("KSCH", "2"))
DVE_TILES = [(), ((1, 1),), ((1, 1), (3, 1)),
             ((1, 0), (1, 1), (3, 1)),
             ((1, 0), (1, 1), (3, 0), (3, 1))][min(_KSCH, 4)]
DVE_TILES = set(DVE_TILES)
K_E3 = os.environ.get("K_E3", "1") == "1"    # K allgather in fp8-e3m4
ROWTILE = os.environ.get("ROWTILE", "1") == "1"
# AV runs KLAG blocks behind scores/exp so its matmuls never head-of-
# line-block the PE queue while the V allgathers are still in flight.
KLAG = int(os.environ.get("KLAG", "3"))


def _build(flags):
    has_bqk, has_bv, has_gb = flags
    K_DT = FP8E3 if K_E3 else BF16
    V_DT = BF16
    nc = bacc.Bacc("TRN2", target_bir_lowering=False, debug=False,
                   num_devices=NC)

    xqT = nc.dram_tensor("xqT", [D, R], BF16, kind="ExternalInput")
    xvT = nc.dram_tensor("xvT", [D, R], BF16, kind="ExternalInput")
    wq_d = nc.dram_tensor("wq", [D, D], BF16, kind="ExternalInput")
    wk_d = nc.dram_tensor("wk", [D, D], BF16, kind="ExternalInput")
    wv_d = nc.dram_tensor("wv", [D, D], BF16, kind="ExternalInput")
    perm_d = nc.dram_tensor("perm", [128, 128], BF16, kind="ExternalInput")
    ident_d = nc.dram_tensor("ident", [128, 128], BF16, kind="ExternalInput")
    cos_d = nc.dram_tensor("cos", [128, R], F32, kind="ExternalInput")
    sin_d = nc.dram_tensor("sin", [128, R], F32, kind="ExternalInput")
    if has_bqk:
        cq_d = nc.dram_tensor("cq", [D, R], F32, kind="ExternalInput")
        ck_d = nc.dram_tensor("ck", [D, R], F32, kind="ExternalInput")
    if has_bv:
        bv_d = nc.dram_tensor("bv", [128, D], F32, kind="ExternalInput")
    if has_gb:
        gam_d = nc.dram_tensor("gamma", [128, D], F32, kind="ExternalInput")
        bet_d = nc.dram_tensor("beta", [128, D], F32, kind="ExternalInput")
    out_d = nc.dram_tensor("out", [R, D], F32, kind="ExternalOutput")

    es = ExitStack()
    with es:
        tc = es.enter_context(tile.TileContext(nc))
        dram = es.enter_context(
            tc.tile_pool(name="dram", bufs=1, space="DRAM"))
        constp = es.enter_context(tc.tile_pool(name="const", bufs=1))
        qp = es.enter_context(tc.tile_pool(name="qp", bufs=1))
        kvs = es.enter_context(tc.tile_pool(name="kvs", bufs=8))
        attnp = es.enter_context(tc.tile_pool(name="attnp", bufs=1))
        epip = es.enter_context(tc.tile_pool(name="epip", bufs=8))
        lnp = es.enter_context(tc.tile_pool(name="lnp", bufs=2))
        outp = es.enter_context(tc.tile_pool(name="outp", bufs=2))

        # K allgather split by batch: batch 0's K lands first and unblocks
        # the (batch-outer-ordered) attention blocks ~10us earlier.
        bounce_kb = [dram.tile([D, SL], K_DT, tag=f"bkb{b}",
                               name=f"bkb{b}") for b in range(B)]
        ag_kb = [dram.tile([NC * D, SL], K_DT, tag=f"agkb{b}",
                           name=f"agkb{b}", addr_space="Shared")
                 for b in range(B)]
        bounce_v = dram.tile([R, H * 65], V_DT, tag="bv")
        # V allgather is split by batch so AV for batch 0 can start while
        # batch 1's shards are still in flight.
        ag_vb = [dram.tile([NC * SL, H * 65], V_DT, tag=f"agv{b}",
                           name=f"agv{b}", addr_space="Shared")
                 for b in range(B)]

        cos_sb = constp.tile([128, R], F32, tag="cos")
        sin_sb = constp.tile([128, R], F32, tag="sin")
        perm_sb = constp.tile([128, 128], BF16, tag="perm")
        ident_sb = constp.tile([128, 128], BF16, tag="ident")
        eps_sb = constp.tile([128, 1], F32, tag="eps")
        nc.vector.memset(eps_sb[:], LN_EPS)
        cq_sb = ck_sb = bv_sb = gam_sb = bet_sb = None
        if has_bqk:
            cq_sb = constp.tile([128, DC * R], F32, tag="cq")
            ck_sb = constp.tile([128, DC * R], F32, tag="ck")
            for g in range(G):
                nc.sync.dma_start(cq_sb[:, g * R:(g + 1) * R],
                                  cq_d[g * 128:(g + 1) * 128, :])
                nc.sync.dma_start(ck_sb[:, g * R:(g + 1) * R],
                                  ck_d[g * 128:(g + 1) * 128, :])
        if has_bv:
            bv_sb = constp.tile([128, D], F32, tag="bvs")
            nc.sync.dma_start(bv_sb[:], bv_d[:])
        if has_gb:
            gam_sb = constp.tile([128, D], F32, tag="gam")
            nc.sync.dma_start(gam_sb[:], gam_d[:])
            bet_sb = constp.tile([128, D], F32, tag="bet")
            nc.sync.dma_start(bet_sb[:], bet_d[:])

        q_sb = qp.tile([128, G * R], BF16, tag="qrot")

        pes = ExitStack()
        with pes:
            xp = pes.enter_context(tc.tile_pool(name="xp", bufs=1))
            wp = pes.enter_context(tc.tile_pool(name="wp", bufs=2))
            stage = pes.enter_context(tc.tile_pool(name="stage", bufs=3))
            usbp = pes.enter_context(tc.tile_pool(name="usbp", bufs=3))
            krotp = pes.enter_context(tc.tile_pool(name="krotp", bufs=2))
            vstp = pes.enter_context(tc.tile_pool(name="vstp", bufs=2))
            pjp = pes.enter_context(
                tc.tile_pool(name="pjp", bufs=4, space="PSUM"))
            pvp = pes.enter_context(
                tc.tile_pool(name="pvp", bufs=2, space="PSUM"))

            def load_w(t_dram):
                w_sb = wp.tile([128, DC * D], BF16, tag="w")
                for dc in range(DC):
                    nc.sync.dma_start(w_sb[:, dc * D:(dc + 1) * D],
                                      t_dram[dc * 128:(dc + 1) * 128, :])
                return w_sb

            # K first: its all-gather is on the attention critical path.
            wk_sb = load_w(wk_d)
            xq_sb = xp.tile([128, DC * R], BF16, tag="xq")
            for dc in range(DC):
                nc.sync.dma_start(xq_sb[:, dc * R:(dc + 1) * R],
                                  xqT[dc * 128:(dc + 1) * 128, :])
            nc.sync.dma_start(perm_sb[:], perm_d[:])
            nc.sync.dma_start(cos_sb[:], cos_d[:])
            nc.sync.dma_start(sin_sb[:], sin_d[:])
            nc.sync.dma_start(ident_sb[:], ident_d[:])
            xv_sb = xp.tile([128, DC * R], BF16, tag="xv")
            for dc in range(DC):
                nc.sync.dma_start(xv_sb[:, dc * R:(dc + 1) * R],
                                  xvT[dc * 128:(dc + 1) * 128, :])

            # --- Q/K projection, software-pipelined so the perm matmul of
            # group g runs behind the U matmuls of group g+1. ---
            def proj_u(w_sb, g):
                ps_u = pjp.tile([128, R], F32, tag="pj",
                                name=f"psu{id(w_sb)}_{g}")
                for dc in range(DC):
                    nc.tensor.matmul(
                        ps_u[:],
                        w_sb[:, dc * D + g * 128: dc * D + (g + 1) * 128],
                        xq_sb[:, dc * R:(dc + 1) * R],
                        start=(dc == 0), stop=(dc == DC - 1))
                u_sb = usbp.tile([128, R], BF16, tag="usb",
                                 name=f"usb{id(w_sb)}_{g}")
                nc.scalar.copy(u_sb[:], ps_u[:])
                return ps_u, u_sb

            def proj_rope(g, ps_u, u_sb, c_sb, dst):
                ps_u2 = pjp.tile([128, R], F32, tag="pj", name=f"psu2_{g}")
                nc.tensor.matmul(ps_u2[:], perm_sb[:], u_sb[:],
                                 start=True, stop=True)
                t1 = stage.tile([128, R], F32, tag="st", name=f"t1_{g}")
                nc.vector.tensor_mul(t1[:], ps_u[:], cos_sb[:])
                t2 = stage.tile([128, R], F32, tag="st", name=f"t2_{g}")
                nc.vector.tensor_mul(t2[:], ps_u2[:], sin_sb[:])
                if c_sb is None:
                    nc.vector.tensor_add(dst, t1[:], t2[:])
                else:
                    t3 = stage.tile([128, R], F32, tag="st", name=f"t3_{g}")
                    nc.vector.tensor_add(t3[:], t1[:], t2[:])
                    nc.vector.tensor_add(
                        dst, t3[:], c_sb[:, g * R:(g + 1) * R])

            def qk_proj_all(w_sb, c_sb, emit_dst, tail, groups):
                pend = None
                for g in groups:
                    cur = (g,) + proj_u(w_sb, g)
                    if pend is not None:
                        gp = pend[0]
                        proj_rope(*pend, c_sb, emit_dst(gp))
                        tail(gp)
                    pend = cur
                gp = pend[0]
                proj_rope(*pend, c_sb, emit_dst(gp))
                tail(gp)

            # K projection + RoPE -> bounce (fp8), single AllGather
            krots = {}

            def k_dst(g):
                krots[g] = krotp.tile([128, R], K_DT, tag="kr",
                                      name=f"kr{g}")
                return krots[g][:]

            def k_tail(g):
                for b in range(B):
                    nc.sync.dma_start(
                        bounce_kb[b][g * 128:(g + 1) * 128, :],
                        krots[g][:, b * SL:(b + 1) * SL])

            qk_proj_all(wk_sb, ck_sb, k_dst, k_tail, range(G))
            nc.gpsimd.collective_compute(
                "AllGather", ALU.bypass,
                ins=[bounce_kb[0][:].opt()], outs=[ag_kb[0][:].opt()],
                replica_groups=[list(range(NC))])

            # V projection -> bounce (fp8, ones interleaved), AllGather
            wv_sb = load_w(wv_d)
            for st in range(R // 128):
                ps_v = pvp.tile([128, D], F32, tag="pv")
                for dc in range(DC):
                    for hf in range(2):
                        nc.tensor.matmul(
                            ps_v[:, hf * 512:(hf + 1) * 512],
                            xv_sb[:, dc * R + st * 128:
                                  dc * R + st * 128 + 128],
                            wv_sb[:, dc * D + hf * 512:
                                  dc * D + (hf + 1) * 512],
                            start=(dc == 0), stop=(dc == DC - 1))
                if has_bv:
                    nc.vector.tensor_add(ps_v[:], ps_v[:], bv_sb[:])
                v_sb = vstp.tile([128, H * 65], V_DT, tag="vst")
                v3 = v_sb[:].rearrange("p (h e) -> p h e", e=65)
                nc.vector.memset(v3[:, :, 64:65], 1.0)
                nc.scalar.copy(
                    v3[:, :, 0:64],
                    ps_v[:].rearrange("p (h d) -> p h d", d=64))
                nc.sync.dma_start(
                    bounce_v[st * 128:(st + 1) * 128, :], v_sb[:])
                if st % 2 == 1:
                    b = st // 2
                    nc.gpsimd.collective_compute(
                        "AllGather", ALU.bypass,
                        ins=[bounce_v[b * SL:(b + 1) * SL, :].opt()],
                        outs=[ag_vb[b][:].opt()],
                        replica_groups=[list(range(NC))])
                    if b == 0:
                        # batch 1's K chains warm between V0 and V1
                        nc.gpsimd.collective_compute(
                            "AllGather", ALU.bypass,
                            ins=[bounce_kb[1][:].opt()],
                            outs=[ag_kb[1][:].opt()],
                            replica_groups=[list(range(NC))])

            # Q projection + RoPE (stays local).
            wq_sb = load_w(wq_d)

            def q_dst(g):
                return q_sb[:, g * R:(g + 1) * R]

            qk_proj_all(wq_sb, cq_sb, q_dst, lambda g: None, range(G))

        kph_all = {}

        def load_pair(g, b):
            # one batch-half of one head-pair's K, reloaded per block
            kph = kvs.tile([128, S], K_DT, tag="kp", name=f"kp{g}_{b}")
            for r in range(NC):
                srcap = ag_kb[b][r * D + g * 128: r * D + (g + 1) * 128, :]
                nc.sync.dma_start(kph[:, r * SL:(r + 1) * SL], srcap)
            kph_all[(g, b)] = kph

        # pts pool is created after the projection pools are released so
        # the KLAG-deep prob tiles reuse that SBUF.
        ptp = es.enter_context(
            tc.tile_pool(name="ptp", bufs=8 * (KLAG + 1) + 2))

        # prefetch block 0's K before the bulk v_full loads hit the queues
        load_pair(0, 0)

        # V resident for the whole attention phase: [s-tile, 16 heads x 65]
        # per (batch, k-tile), contiguous lines.  The loads are emitted
        # lazily inside the block loop (load_v below) so the descriptors
        # don't sit in the DMA queues blocking per-block K loads while
        # the V allgather is still in flight.
        vfp = es.enter_context(tc.tile_pool(name="vfp", bufs=1))
        v_full = vfp.tile([128, B * KT * H * 65], V_DT, tag="vfull")

        def load_v(b, kt):
            nc.sync.dma_start(
                v_full[:, (b * KT + kt) * (H * 65):
                       (b * KT + kt + 1) * (H * 65)],
                ag_vb[b][kt * 128:(kt + 1) * 128, :])

        attn_sb = [attnp.tile([128, D], F32, tag=f"attn{t}", name=f"attn{t}")
                   for t in range(4)]

        aes = ExitStack()
        with aes:
            scp = aes.enter_context(
                tc.tile_pool(name="scp", bufs=3, space="PSUM"))
            avp = aes.enter_context(
                tc.tile_pool(name="avp", bufs=1, space="PSUM"))
            trp = aes.enter_context(
                tc.tile_pool(name="trp", bufs=1, space="PSUM"))
            atsb = aes.enter_context(tc.tile_pool(name="atsb", bufs=3))
            schp = aes.enter_context(tc.tile_pool(name="schp", bufs=2))

            def emit_scores(g, b, grp):
                kph = kph_all[(g, b)]
                ps_s = [scp.tile([128, 1024], F32, tag="sc",
                                 name=f"pss{g}_{b}_{grp}_{_i}")
                        for _i in range(2)]
                for jj in range(4):
                    kt = grp * 4 + jj
                    for hh in range(2):
                        nc.tensor.matmul(
                            ps_s[hh][:, jj * SL:(jj + 1) * SL],
                            kph[hh * 64:(hh + 1) * 64,
                                kt * 128:(kt + 1) * 128],
                            q_sb[hh * 64:(hh + 1) * 64,
                                 g * R + b * SL:
                                 g * R + (b + 1) * SL],
                            start=True, stop=True,
                            tile_position=(hh * 64, 0) if ROWTILE
                            else None)
                return ps_s

            def emit_exp(g, b, grp, hh, ps, pts):
                if (grp, hh) in DVE_TILES:
                    e1 = schp.tile([128, 1024], I16, tag="e1",
                                   name=f"e1_{g}_{b}_{grp}_{hh}")
                    nc.vector.tensor_scalar(
                        e1[:], ps[:], SCH_A, SCH_B1, ALU.mult, ALU.add)
                    e2 = schp.tile([128, 1024], I16, tag="e2",
                                   name=f"e2_{g}_{b}_{grp}_{hh}")
                    nc.vector.tensor_scalar(
                        e2[:], e1[:], SCH_OFF, None, ALU.subtract)
                    pt = ptp.tile([128, 1024], BF16, tag="pt",
                                  name=f"pt{g}_{b}_{grp}_{hh}")
                    nc.vector.tensor_add(
                        pt[:], e1[:].bitcast(BF16), e2[:].bitcast(BF16))
                else:
                    pt = ptp.tile([128, 1024], BF16, tag="pt",
                                  name=f"pt{g}_{b}_{grp}_{hh}")
                    nc.scalar.activation(
                        pt[:], ps[:], AF.Exp, scale=0.125)
                pts[(grp, hh)] = pt[:]

            def emit_av_quarter(g, b, pts, aTp, grp):
                # attn^T accumulation: out[65, 256] = [V_h | 1]^T @ P^T.
                # start=True clears has_written for the WHOLE bank, so it
                # may only appear on the block's very first AV matmul;
                # the bank-wide clear lets every later matmul of both
                # head-halves overwrite-then-accumulate correctly.
                for hh in range(2):
                    h = 2 * g + hh
                    aT = aTp[:, hh * SL:(hh + 1) * SL]
                    for jj in range(4):
                        kt = grp * 4 + jj
                        nc.tensor.matmul(
                            aT,
                            v_full[:, (b * KT + kt) * (H * 65)
                                   + h * 65:
                                   (b * KT + kt) * (H * 65)
                                   + (h + 1) * 65],
                            pts[(grp, hh)][:, jj * SL:(jj + 1) * SL],
                            start=(kt == 0 and hh == 0),
                            stop=(kt == 15 and hh == 1),
                            skip_group_check=True)

            def emit_cast(g, b, aTp):
                aT_sb = atsb.tile([65, 2 * SL], BF16, tag="ats",
                                  name=f"ats{g}_{b}")
                nc.vector.tensor_copy(aT_sb[:], aTp[:])
                return aT_sb

            def emit_tr_norm(g, b, aT_sb):
                # PE-transpose attn^T back to [q, dh], then normalize by
                # the gathered denominators (65th row).
                tr = trp.tile([128, 4 * 66], BF16, tag="tr",
                              name=f"tr{g}_{b}")
                for hh in range(2):
                    for t in range(2):
                        idx = hh * 2 + t
                        nc.tensor.transpose(
                            tr[:, idx * 66: idx * 66 + 65],
                            aT_sb[:, hh * SL + t * 128:
                                  hh * SL + (t + 1) * 128],
                            ident_sb[0:65, 0:65])
                rec = epip.tile([128, 4], F32, tag="rec",
                                name=f"rec{g}_{b}")
                nc.vector.reciprocal(rec[:], tr[:, 64::66])
                for hh in range(2):
                    h = 2 * g + hh
                    for t in range(2):
                        idx = hh * 2 + t
                        qtg = b * 2 + t
                        nc.vector.tensor_scalar(
                            attn_sb[qtg][:, h * 64:(h + 1) * 64],
                            tr[:, idx * 66: idx * 66 + 64],
                            rec[:, idx: idx + 1], None, ALU.mult)

            def emit_fixups(g, b, aTp):
                emit_tr_norm(g, b, emit_cast(g, b, aTp))

            # KLAG-block software pipeline with quarter-grain interleave:
            # the AV matmuls of block i-KLAG are emitted between the
            # score chunk-pairs of block i, so the PE always has
            # independent, dependency-satisfied work queued while the exp
            # stream paces the pipeline (and the HAM clock gate stays
            # warm).  Batch-outer block order: the V allgather for batch
            # b lands well before block b*G+KLAG needs it.
            blocks = [(g, b) for b in range(B) for g in range(G)]
            pend = []   # (g, b, pts) awaiting AV, oldest first

            def do_av(g, b, pts):
                aTp = avp.tile([65, 2 * SL], F32, tag="av",
                               name=f"aT{g}_{b}")
                for grp in range(4):
                    emit_av_quarter(g, b, pts, aTp, grp)
                return aTp

            fixq = []   # (g, b, aT_sb) cast done, awaiting transpose+norm
            for i, (g, b) in enumerate(blocks):
                fix_old = fixq.pop(0) if fixq else None
                if i + 1 < len(blocks):
                    load_pair(*blocks[i + 1])
                # stream the v_full loads: batch 0's over blocks 0-3
                # (complete before their first AV consumer at block KLAG),
                # batch 1's over blocks 7-10 (emitted before their block-
                # (8+KLAG) consumer but late enough that the descriptors
                # don't sit in the DMA queues waiting on the V1 allgather)
                if i < 4:
                    for kt in range(4 * i, 4 * i + 4):
                        load_v(0, kt)
                elif 7 <= i < 11:
                    for kt in range(4 * (i - 7), 4 * (i - 7) + 4):
                        load_v(1, kt)
                pts = {}
                aT_prev = None
                old = pend.pop(0) if len(pend) >= KLAG else None
                for grp in range(4):
                    ps_s = emit_scores(g, b, grp)
                    if old is not None:
                        if grp == 0:
                            aT_prev = avp.tile(
                                [65, 2 * SL], F32, tag="av",
                                name=f"aT{old[0]}_{old[1]}")
                        emit_av_quarter(old[0], old[1], old[2],
                                        aT_prev, grp)
                    if grp == 2 and fix_old is not None:
                        # one block after its AV: PE transposes + DVE
                        # normalize, with inputs long since ready (no
                        # head-of-line stalls in any engine queue)
                        emit_tr_norm(fix_old[0], fix_old[1], fix_old[2])
                    emit_exp(g, b, grp, 0, ps_s[0], pts)
                    emit_exp(g, b, grp, 1, ps_s[1], pts)
                if old is not None:
                    # evacuate aT promptly (frees the single avp buffer
                    # for the next block's AV quarters)
                    fixq.append((old[0], old[1],
                                 emit_cast(old[0], old[1], aT_prev)))
                # drain an extra pending AV near the end so the tail after
                # the last exp is short
                if i >= len(blocks) - KLAG + 1 and pend:
                    g2, b2, pts2 = pend.pop(0)
                    aTp2 = do_av(g2, b2, pts2)
                    fixq.append((g2, b2, emit_cast(g2, b2, aTp2)))
                pend.append((g, b, pts))
            for (g, b, pts) in pend:
                aTp = do_av(g, b, pts)
                fixq.append((g, b, emit_cast(g, b, aTp)))
            for (g, b, aT_sb) in fixq:
                emit_tr_norm(g, b, aT_sb)

        # --- LayerNorm over D (var = E[x^2] - mu^2; square+row-sum on the
        # ScalarE accumulator) + store ---
        for qtg in range(4):
            at = attn_sb[qtg]
            sums = epip.tile([128, 1], F32, tag="s1", name=f"s1_{qtg}")
            nc.vector.reduce_sum(sums[:], at[:], axis=AX.X)
            sq = lnp.tile([128, D], F32, tag="sq", name=f"sq{qtg}")
            ssum = epip.tile([128, 1], F32, tag="s3", name=f"s3_{qtg}")
            nc.scalar.activation(sq[:], at[:], AF.Square,
                                 accum_out=ssum[:])
            mu = epip.tile([128, 1], F32, tag="s2", name=f"s2_{qtg}")
            nc.vector.tensor_scalar_mul(mu[:], sums[:], 1.0 / D)
            var = epip.tile([128, 1], F32, tag="s6", name=f"s6_{qtg}")
            nc.vector.tensor_scalar(var[:], mu[:], mu[:], None, ALU.mult)
            nc.vector.tensor_scalar(
                var[:], var[:], -1.0, None, ALU.mult)
            nc.vector.scalar_tensor_tensor(
                var[:], ssum[:], 1.0 / D, var[:], ALU.mult, ALU.add)
            std = epip.tile([128, 1], F32, tag="s4", name=f"s4_{qtg}")
            nc.scalar.activation(std[:], var[:], AF.Sqrt, bias=eps_sb[:])
            rstd = epip.tile([128, 1], F32, tag="s5", name=f"s5_{qtg}")
            nc.vector.reciprocal(rstd[:], std[:])
            mrs = epip.tile([128, 1], F32, tag="s7", name=f"s7_{qtg}")
            nc.vector.tensor_scalar(mrs[:], mu[:], rstd[:], None, ALU.mult)
            o_sb = outp.tile([128, D], F32, tag="o", name=f"oo{qtg}")
            nc.vector.tensor_scalar(
                o_sb[:], at[:], rstd[:], mrs[:], ALU.mult, ALU.subtract)
            if has_gb:
                nc.vector.tensor_mul(o_sb[:], o_sb[:], gam_sb[:])
                nc.vector.tensor_add(o_sb[:], o_sb[:], bet_sb[:])
            nc.sync.dma_start(out_d[qtg * 128:(qtg + 1) * 128, :], o_sb[:])

    nc.compile()
    return nc


_CACHE: dict = {}
LAST_EXEC_NS = None


def _rope_tables():
    half = DH // 2
    inv_freq = 1.0 / (ROPE_BASE ** (np.arange(half, dtype=np.float32) / half))
    t = np.arange(S, dtype=np.float32)
    freqs = t[:, None] * inv_freq[None, :]
    emb = np.concatenate([freqs, freqs], axis=-1)          # [S, DH]
    return np.cos(emb).astype(np.float32), np.sin(emb).astype(np.float32)


def prep_flags(inputs):
    b_qk = np.asarray(inputs["b_qk"], dtype=np.float32)
    b_v = np.asarray(inputs["b_v"], dtype=np.float32)
    gamma = np.asarray(inputs["ln_gamma"], dtype=np.float32)
    beta = np.asarray(inputs["ln_beta"], dtype=np.float32)
    return (bool(np.any(b_qk)), bool(np.any(b_v)),
            bool(np.any(gamma != 1.0) or np.any(beta != 0.0)))


def _prep_in_maps(inputs, flags):
    x_qk = np.asarray(inputs["x_qk"], dtype=np.float32)
    x_v = np.asarray(inputs["x_v"], dtype=np.float32)
    W_qk = np.asarray(inputs["W_qk"], dtype=np.float32)
    b_qk = np.asarray(inputs["b_qk"], dtype=np.float32)
    W_v = np.asarray(inputs["W_v"], dtype=np.float32)
    b_v = np.asarray(inputs["b_v"], dtype=np.float32)
    gamma = np.asarray(inputs["ln_gamma"], dtype=np.float32)
    beta = np.asarray(inputs["ln_beta"], dtype=np.float32)

    # signed pair-swap: rot2(v)[j] = sum_l Pm[l, j] v[l]
    Pm = np.zeros((128, 128), np.float32)
    for i in range(64):
        Pm[2 * i + 1, 2 * i] = -1.0
        Pm[2 * i, 2 * i + 1] = 1.0
    Pm64 = Pm[:DH, :DH]

    cos_all, sin_all = _rope_tables()
    Wq = W_qk[:, :D]
    Wk = W_qk[:, D:]
    bq = b_qk[:D]
    bk = b_qk[D:]
    bq2 = (bq.reshape(H, DH) @ Pm64).reshape(D)
    bk2 = (bk.reshape(H, DH) @ Pm64).reshape(D)

    wq_np = np.ascontiguousarray(Wq.astype(NP_BF16))
    wk_np = np.ascontiguousarray(Wk.astype(NP_BF16))
    wv_np = np.ascontiguousarray(W_v.astype(NP_BF16))
    perm_np = np.ascontiguousarray(Pm.astype(NP_BF16))

    xf = x_qk.reshape(B * S, D)
    xvf = x_v.reshape(B * S, D)

    in_maps = []
    for c in range(NC):
        ps = np.arange(SL * c, SL * (c + 1))
        rows = np.concatenate([ps, S + ps])          # both batches
        xqT_c = np.ascontiguousarray(xf[rows].T.astype(NP_BF16))
        xvT_c = np.ascontiguousarray(xvf[rows].T.astype(NP_BF16))
        cos_c = np.ascontiguousarray(np.tile(cos_all[ps].T, (2, 2)))
        sin_c = np.ascontiguousarray(np.tile(sin_all[ps].T, (2, 2)))
        m = {
            "xqT": xqT_c, "xvT": xvT_c,
            "wq": wq_np, "wk": wk_np, "wv": wv_np,
            "perm": perm_np, "cos": cos_c, "sin": sin_c,
            "ident": np.ascontiguousarray(np.eye(128, dtype=NP_BF16)),
        }
        if flags[0]:
            cos_f = np.tile(cos_all[ps].T, (H, 2))   # [1024, 512]
            sin_f = np.tile(sin_all[ps].T, (H, 2))
            m["cq"] = np.ascontiguousarray(
                bq[:, None] * cos_f + bq2[:, None] * sin_f)
            m["ck"] = np.ascontiguousarray(
                bk[:, None] * cos_f + bk2[:, None] * sin_f)
        if flags[1]:
            m["bv"] = np.ascontiguousarray(
                np.broadcast_to(b_v, (128, D)).astype(np.float32))
        if flags[2]:
            m["gamma"] = np.ascontiguousarray(
                np.broadcast_to(gamma, (128, D)).astype(np.float32))
            m["beta"] = np.ascontiguousarray(
                np.broadcast_to(beta, (128, D)).astype(np.float32))
        in_maps.append(m)
    return in_maps


def assemble_output(per_core_outs):
    out = np.empty((B * S, D), np.float32)
    for c in range(NC):
        oc = np.asarray(per_core_outs[c], dtype=np.float32)
        for b in range(B):
            out[b * S + SL * c: b * S + SL * (c + 1)] = \
                oc[b * SL:(b + 1) * SL]
    return out.reshape(B, S, D)


def kernel(**inputs):
    flags = prep_flags(inputs)
    if flags not in _CACHE:
        _CACHE[flags] = _build(flags)
    nc = _CACHE[flags]
    in_maps = _prep_in_maps(inputs, flags)
    res = bass_utils.run_bass_kernel_spmd(
        nc, in_maps, core_ids=list(range(NC)))
    global LAST_EXEC_NS
    LAST_EXEC_NS = res.exec_time_ns
    return assemble_output([res.results[c]["out"] for c in range(NC)])
